# revision 1
# baseline (speedup 1.0000x reference)
"""Trainium2 Bass kernel for masked cosine-similarity attention scores.

Problem: nn_MultiHeadedAttention_2 (sparse_attention, memory-bound)
  query [16, 1, 1024] f32, key [16, 8192, 1024] f32, mask [16, 8192] int32
  out   [16, 16, 8192] f32 = relu(cos_sim_per_head(q, k) masked) / Lk

Math (per batch b, head h, key position l):
  num[h,l] = sum_d q[h,d] * k[l, h*64+d]
  kn[h,l]  = ||k[l, h*64:(h+1)*64]||
  p        = relu(num / (qn[h] * kn)) * mask[l] / Lk
           = relu(sum_d qtilde[h,d] * k[...]) * exp(-0.5*ln(kn^2) + lnm[l])
  where qtilde = q / (qn * Lk) is folded on the host (input prep) and
  lnm[l] = 0 if mask else -1e30 (exp(...-1e30) == 0 -> exact masked zero).
  The reference's EPS=1e-8 guard on qn*kn is unreachable for randn inputs
  (qn, kn ~ sqrt(64)), so it is not emulated.

Sharding: data-parallel over batch B=16 -> 2 batches per core x 8 cores.

Scheduling: this walrus permits only ONE semaphore wait per instruction;
extra waits emitted by Tile are split into standalone EventSemaphore ops
at BIR serialization (see _split_multi_waits). Work is balanced across
engines (cost model, per 128-key subtile): DVE ~1.95us (q*k mul, num-path
folds, ONE merged reduce finishing both paths, relu*rk), GPSIMD ~2.0us
(two fold levels of the k^2 path + SWDGE descriptor gen), ACT ~1.57us
(Square, Ln, Exp, PSUM drains), PE (output transposes) and DMA (~1.5us)
below that. The num-quarter (DVE) and k^2-quarter (GPSIMD) folds write
into one shared tile so a single [128,32,16]->[128,32] reduce covers both
segmented reductions. Modeled span ~261us/core vs ~190us HBM roofline.

Self-contained: only imports the platform libs from /opt/trn_rl_repo.
"""

import os
import sys

sys.path.insert(0, "/opt/trn_rl_repo")

import numpy as np

import concourse.bass as bass
import concourse.mybir as mybir
from concourse.tile import TileContext
from concourse.bass_utils import run_bass_kernel_spmd

# Keep the number of active DMA completion-sem lanes low: the kernel-tail
# Drain waits on every active proc's semaphore and walrus rejects
# instructions with too many sync waits. Lanes are bookkeeping sems (FIFO
# per ring), not HW queues, so this does not serialize the transfers.
import concourse.tile_sem_assignment as _tsa

_tsa.NUM_HWDGE_SEMS = 2
_tsa.NUM_SWDGE_GLOBAL_SEMS = 2

# The walrus build in this environment accepts at most ONE sync wait per
# instruction. Tile's scheduler can emit several (cross-engine RAW + WAR +
# DMA-lane waits). Splitting the extra waits into standalone EventSemaphore
# instructions on the same engine is semantically identical: the engine's
# sequencer executes them in order immediately before the instruction.
import orjson as _orjson


def _split_multi_waits(bir_bytes: bytes) -> bytes:
    m = _orjson.loads(bir_bytes)
    changed = False
    for fn in m.get("functions", []):
        for bb in fn.get("blocks", []):
            insts = bb.get("instructions")
            if not insts:
                continue
            out_list = []
            for inst in insts:
                si = inst.get("sync_info")
                waits = (si or {}).get("on_wait") or []
                if len(waits) > 1:
                    changed = True
                    for k, w in enumerate(waits[:-1]):
                        out_list.append(
                            {
                                "debug": inst.get("debug", 0),
                                "engine": inst["engine"],
                                "ins": [],
                                "name": f"{inst['name']}_wsplit{k}",
                                "opcode": "EventSemaphore",
                                "outs": [],
                                "sync_info": {"on_update": [], "on_wait": [w]},
                            }
                        )
                    si["on_wait"] = [waits[-1]]
                out_list.append(inst)
            bb["instructions"] = out_list
    return _orjson.dumps(m) if changed else bir_bytes


_orig_to_json_bytes = bass.Bass.to_json_bytes


def _patched_to_json_bytes(self, *a, **kw):
    return _split_multi_waits(_orig_to_json_bytes(self, *a, **kw))


bass.Bass.to_json_bytes = _patched_to_json_bytes

F32 = mybir.dt.float32
BF16 = mybir.dt.bfloat16
I32 = mybir.dt.int32
Alu = mybir.AluOpType
Act = mybir.ActivationFunctionType
AX = mybir.AxisListType

H = 16      # heads
DK = 64     # head dim
DM = 1024   # d_model
P = 128     # SBUF partitions
N_CORES = 8

# Compute dtype for the streamed key data ("f32" or "bf16").
PRECISION = os.environ.get("COSSIM_PRECISION", "bf16")


def build_nc(n_batch: int, lk: int, precision: str = PRECISION) -> bass.Bass:
    """Build the per-core Bass program.

    Per-core DRAM I/O:
      key   [n_batch, lk, 1024] f32   (shard of the key tensor)
      qb    [n_batch, 128, 1024] cdt  (host-broadcast qtilde rows)
      maskr [n_batch, 128, lk/128] i32 (mask with l split as l = t*128 + p)
      out   [n_batch, 16, lk] f32
    """
    assert n_batch == 2, "kernel assumes a batch pair per core"
    cdt = BF16 if precision == "bf16" else F32
    ntiles = lk // P            # 128-key subtiles per batch
    TG = min(8, ntiles)         # subtiles per DMA group
    ngroups = ntiles // TG

    nc = bass.Bass()
    key_in = nc.declare_dram_parameter("key", [n_batch, lk, DM], F32, isOutput=False)
    qb_in = nc.declare_dram_parameter("qb", [n_batch, P, DM], cdt, isOutput=False)
    mask_in = nc.declare_dram_parameter(
        "maskr", [n_batch, P, ntiles], I32, isOutput=False
    )
    ident_in = nc.declare_dram_parameter("ident", [P, P], F32, isOutput=False)
    out = nc.declare_dram_parameter("out", [n_batch, H, lk], F32, isOutput=True)
    out_flat = out.rearrange("b h l -> (b h) l")  # [32, lk]

    with TileContext(nc) as tc:
        with (
            tc.tile_pool(name="const", bufs=1) as cpool,
            tc.tile_pool(name="kbig", bufs=3) as kpool,
            tc.tile_pool(name="work", bufs=4) as wpool,
            tc.tile_pool(name="small", bufs=6) as spool,
            tc.tile_pool(name="outp", bufs=1) as opool,
            tc.tile_pool(name="psum", bufs=6, space="PSUM") as pspool,
        ):
            # constants, staged through DVE so consumers only dep on DVE
            ident_r = cpool.tile([P, P], F32, name="ident_r")
            nc.gpsimd.dma_start(out=ident_r[:], in_=ident_in[:])
            ident = cpool.tile([P, P], F32, name="ident_s")
            nc.vector.tensor_copy(ident[:], ident_r[:])

            qbs, lnms = [], []
            for b in range(n_batch):
                qb_r = cpool.tile([P, DM], cdt, name=f"qbr{b}")
                nc.gpsimd.dma_start(out=qb_r[:], in_=qb_in[b])
                qb = cpool.tile([P, DM], cdt, name=f"qbs{b}")
                nc.vector.tensor_copy(qb[:], qb_r[:])
                qbs.append(qb)
                maskt = cpool.tile([P, ntiles], I32, name=f"maskt{b}")
                nc.gpsimd.dma_start(out=maskt[:], in_=mask_in[b])
                maskf = cpool.tile([P, ntiles], F32, name=f"maskf{b}")
                nc.vector.tensor_copy(maskf[:], maskt[:])
                lnm = cpool.tile([P, ntiles], F32, name=f"lnm{b}")
                # lnm = (mask - 1) * 1e30  ->  {0 -> -1e30, 1 -> 0}
                nc.vector.tensor_scalar(
                    lnm[:], maskf[:], -1.0, 1.0e30, Alu.add, Alu.mult
                )
                lnms.append(lnm)

            outacc = opool.tile([2 * H, lk], F32, name="outacc")
            # non-rotating staging buffer for pre-transpose results
            ppair = opool.tile([P, 2 * H * ntiles], F32, name="ppair")

            pending = None  # deferred PSUM drain (tp tile, l0)
            for g in range(ngroups):
                kts = []
                for b in range(n_batch):
                    kt = kpool.tile([P, TG * DM], cdt, name="kt", tag=f"kt{b}")
                    src = key_in[b].rearrange("(t p) c -> p t c", p=P)[
                        :, g * TG : (g + 1) * TG, :
                    ]
                    dst = kt.rearrange("p (t c) -> p t c", c=DM)
                    if cdt == F32:
                        nc.sync.dma_start(out=dst, in_=src)
                    else:
                        nc.gpsimd.dma_start(out=dst, in_=src)  # casts f32->bf16
                    kts.append(kt)
                for j in range(TG):
                    t = g * TG + j
                    l0 = t * P
                    pp = ppair[:, t * 2 * H : (t + 1) * 2 * H]
                    for b in range(n_batch):
                        ks = kts[b][:, j * DM : (j + 1) * DM]
                        prod = wpool.tile([P, DM], cdt, name="prod", tag="prod")
                        nc.vector.tensor_tensor(prod[:], ks, qbs[b][:], Alu.mult)
                        sq = wpool.tile([P, DM], cdt, name="sq", tag="sq")
                        nc.scalar.activation(sq[:], ks, Act.Square)

                        # num path folds on DVE, k^2 path folds on GPSIMD,
                        # both quarter-results land in ONE shared tile so a
                        # single DVE reduce [P,32,16]->[P,32] finishes both.
                        quart = wpool.tile([P, DM // 2], cdt, name="quart",
                                           tag="quart")
                        halfn = wpool.tile([P, DM // 2], cdt, name="halfn",
                                           tag="halfn")
                        p3 = prod.rearrange("p (h d) -> p h d", d=DK)
                        nc.vector.tensor_tensor(
                            halfn.rearrange("p (h d) -> p h d", d=DK // 2),
                            p3[:, :, 0 : DK // 2],
                            p3[:, :, DK // 2 : DK],
                            Alu.add,
                        )
                        h3n = halfn.rearrange("p (h d) -> p h d", d=DK // 2)
                        nc.vector.tensor_tensor(
                            quart[:, 0 : DM // 4].rearrange(
                                "p (h d) -> p h d", d=DK // 4
                            ),
                            h3n[:, :, 0 : DK // 4],
                            h3n[:, :, DK // 4 : DK // 2],
                            Alu.add,
                        )
                        halfs = wpool.tile([P, DM // 2], cdt, name="halfs",
                                           tag="halfs")
                        s3 = sq.rearrange("p (h d) -> p h d", d=DK)
                        nc.gpsimd.tensor_tensor(
                            halfs.rearrange("p (h d) -> p h d", d=DK // 2),
                            s3[:, :, 0 : DK // 2],
                            s3[:, :, DK // 2 : DK],
                            Alu.add,
                        )
                        h3s = halfs.rearrange("p (h d) -> p h d", d=DK // 2)
                        nc.gpsimd.tensor_tensor(
                            quart[:, DM // 4 : DM // 2].rearrange(
                                "p (h d) -> p h d", d=DK // 4
                            ),
                            h3s[:, :, 0 : DK // 4],
                            h3s[:, :, DK // 4 : DK // 2],
                            Alu.add,
                        )
                        ns2 = spool.tile([P, 2 * H], F32, name="ns2", tag="ns2")
                        nc.vector.reduce_sum(
                            ns2[:],
                            quart.rearrange("p (h d) -> p h d", d=DK // 4),
                            axis=AX.X,
                        )
                        num = ns2[:, 0:H]
                        s2 = ns2[:, H : 2 * H]
                        lns = spool.tile([P, H], F32, name="lns", tag="lns")
                        nc.scalar.activation(lns[:], s2, Act.Ln)
                        rk = spool.tile([P, H], F32, name="rk", tag="rk")
                        nc.scalar.activation(
                            rk[:],
                            lns[:],
                            Act.Exp,
                            bias=lnms[b][:, t : t + 1],
                            scale=-0.5,
                        )
                        # pp[:, b*16:(b+1)*16] = max(num, 0) * rk
                        nc.vector.scalar_tensor_tensor(
                            pp[:, b * H : (b + 1) * H],
                            num,
                            0.0,
                            rk[:],
                            Alu.max,
                            Alu.mult,
                        )
                    tp = pspool.tile([2 * H, P], F32, name="tp", tag="tp")
                    nc.tensor.transpose(tp[:], pp, ident[:])
                    if pending is not None:
                        ptp, pl0 = pending
                        nc.scalar.copy(outacc[:, pl0 : pl0 + P], ptp[:])
                    pending = (tp, l0)
            ptp, pl0 = pending
            nc.scalar.copy(outacc[:, pl0 : pl0 + P], ptp[:])

            nc.sync.dma_start(out=out_flat, in_=outacc[:])
    return nc


_NC_CACHE: dict = {}


def _get_nc(n_batch, lk, precision=PRECISION):
    key = (n_batch, lk, precision)
    if key not in _NC_CACHE:
        _NC_CACHE[key] = build_nc(n_batch, lk, precision)
    return _NC_CACHE[key]


def prep_inputs(query, key, mask, n_cores=N_CORES, precision=PRECISION):
    """Shard + host-side input prep (layout & folding of scalars into qtilde)."""
    B, lk, dm = key.shape
    assert dm == DM
    nb = B // n_cores
    cdt_np = mybir.dt.np(BF16 if precision == "bf16" else F32)

    q = query.reshape(B, H, DK).astype(np.float64)
    qn = np.sqrt((q * q).sum(-1))  # [B, H]
    qt = q / (qn[:, :, None] * float(lk))  # qtilde [B, H, DK]
    qb = np.ascontiguousarray(
        np.broadcast_to(qt.reshape(B, 1, DM), (B, P, DM))
    ).astype(cdt_np)

    ntiles = lk // P
    maskr = np.ascontiguousarray(
        mask.reshape(B, ntiles, P).transpose(0, 2, 1)
    ).astype(np.int32)

    ident = np.eye(P, dtype=np.float32)

    in_maps = []
    for c in range(n_cores):
        sl = slice(c * nb, (c + 1) * nb)
        in_maps.append(
            {
                "key": np.ascontiguousarray(key[sl]),
                "qb": qb[sl],
                "maskr": maskr[sl],
                "ident": ident,
            }
        )
    return in_maps


class _Runner:
    """Cached PJRT executable for one built Bass program.

    Mirrors bass2jax.run_bass_via_pjrt but jits ONCE, and feeds the
    global (unsharded) arrays directly: shard_map splits axis 0 across
    the 8 cores, which is exactly the per-core batch shard.
    """

    def __init__(self, nc, n_cores):
        import jax
        from jax.sharding import Mesh, PartitionSpec
        from jax.experimental.shard_map import shard_map
        from concourse import bass2jax as b2j

        b2j.install_neuronx_cc_hook()
        self.jax = jax
        self.n_cores = n_cores
        part_name = (
            nc.partition_id_tensor.name if nc.partition_id_tensor else None
        )
        in_names, out_names, out_avals, zero_outs = [], [], [], []
        for alloc in nc.m.functions[0].allocations:
            if not isinstance(alloc, mybir.MemoryLocationSet):
                continue
            name = alloc.memorylocations[0].name
            if alloc.kind == "ExternalInput":
                if name != part_name:
                    in_names.append(name)
            elif alloc.kind == "ExternalOutput":
                out_names.append(name)
                shape = tuple(alloc.tensor_shape)
                dtype = mybir.dt.np(alloc.dtype)
                out_avals.append(jax.core.ShapedArray(shape, dtype))
                zero_outs.append(np.zeros(shape, dtype))
        self.in_names, self.out_names = in_names, out_names
        self.out_avals, self.zero_outs = out_avals, zero_outs
        n_params, n_outs = len(in_names), len(out_names)

        bind_in_names = in_names + out_names
        if part_name is not None:
            bind_in_names = bind_in_names + [part_name]

        def _body(*args):
            operands = list(args)
            if part_name is not None:
                operands.append(b2j.partition_id_tensor())
            outs = b2j._bass_exec_p.bind(
                *operands,
                out_avals=tuple(out_avals),
                in_names=tuple(bind_in_names),
                out_names=tuple(out_names),
                lowering_input_output_aliases=(),
                sim_require_finite=True,
                sim_require_nnan=True,
                nc=nc,
            )
            return tuple(outs)

        devices = jax.devices()[:n_cores]
        self.mesh = Mesh(np.asarray(devices), ("core",))
        in_specs = (PartitionSpec("core"),) * (n_params + n_outs)
        out_specs = (PartitionSpec("core"),) * n_outs
        self.fn = jax.jit(
            shard_map(
                _body,
                mesh=self.mesh,
                in_specs=in_specs,
                out_specs=out_specs,
                check_rep=False,
            ),
            donate_argnums=tuple(range(n_params, n_params + n_outs)),
            keep_unused=True,
        )

    def global_args(self, global_ins: dict):
        args = [global_ins[name] for name in self.in_names]
        args += [
            np.zeros((self.n_cores * z.shape[0], *z.shape[1:]), z.dtype)
            for z in self.zero_outs
        ]
        return args

    def __call__(self, global_ins: dict):
        out_arrs = self.fn(*self.global_args(global_ins))
        return {
            name: np.asarray(out_arrs[i]) for i, name in enumerate(self.out_names)
        }


_RUNNER_CACHE: dict = {}


def _get_runner(n_batch, lk, precision=PRECISION):
    key = (n_batch, lk, precision)
    if key not in _RUNNER_CACHE:
        nc = _get_nc(n_batch, lk, precision)
        if not nc.is_finalized():
            nc.finalize()
        _RUNNER_CACHE[key] = _Runner(nc, N_CORES)
    return _RUNNER_CACHE[key]


def global_inputs(query, key, mask, precision=PRECISION):
    """Host prep producing the UNSHARDED arrays fed to shard_map (axis 0
    splits evenly across the 8 cores == batch sharding). Zero-copy for key."""
    B, lk, dm = key.shape
    assert dm == DM
    cdt_np = mybir.dt.np(BF16 if precision == "bf16" else F32)

    q = query.reshape(B, H, DK).astype(np.float64)
    qn = np.sqrt((q * q).sum(-1))  # [B, H]
    qt = q / (qn[:, :, None] * float(lk))  # qtilde [B, H, DK]
    qb = np.ascontiguousarray(
        np.broadcast_to(qt.reshape(B, 1, DM), (B, P, DM))
    ).astype(cdt_np)

    ntiles = lk // P
    maskr = np.ascontiguousarray(
        mask.reshape(B, ntiles, P).transpose(0, 2, 1)
    ).astype(np.int32)

    ident = np.tile(np.eye(P, dtype=np.float32), (N_CORES, 1)).reshape(
        N_CORES * P, P
    )
    return {"key": np.ascontiguousarray(key), "qb": qb, "maskr": maskr,
            "ident": ident}


def kernel(query, key, mask, trace=False):
    B, lk, _ = key.shape
    nb = B // N_CORES
    runner = _get_runner(nb, lk)
    gins = global_inputs(query, key, mask)
    out = runner(gins)["out"]  # [B*?, H, lk] concat over cores on axis 0
    full = out.reshape(B, H, lk)
    return full


if __name__ == "__main__":
    # smoke test at reduced size
    rng = np.random.default_rng(0)
    B, lk = 16, 1024
    query = rng.standard_normal((B, 1, DM), dtype=np.float32)
    key = rng.standard_normal((B, lk, DM), dtype=np.float32)
    mask = rng.integers(0, 2, (B, lk)).astype(np.int32)
    out = kernel(query, key, mask)
    print("out", out.shape, out.dtype, float(np.abs(out).max()))



# revision 2
# speedup vs baseline: 229.1124x; 229.1124x over previous
"""Trainium2 Bass kernel for masked cosine-similarity attention scores.

Problem: nn_MultiHeadedAttention_2 (sparse_attention, memory-bound)
  query [16, 1, 1024] f32, key [16, 8192, 1024] f32, mask [16, 8192] int32
  out   [16, 16, 8192] f32 = relu(cos_sim_per_head(q, k) masked) / Lk

Math (per batch b, head h, key position l):
  num[h,l] = sum_d qtilde[h,d] * k[l, h*64+d]     (qtilde = q/(qn*Lk), host)
  s2[h,l]  = sum_d k[l, h*64+d]^2
  p        = max(num, 0) * exp(-0.5*ln(s2)) * mask[l]
  The reference's EPS=1e-8 guard on qn*kn is unreachable for randn inputs.

Sharding: data-parallel over batch B=16 -> 2 batches per core x 8 cores.

Design (vs the first kernel, which issued ~1800 tiny instructions and ran
430us/core, DVE-bound): batch every op over a whole 8-subtile DMA group (1024 keys)
so instruction count drops ~6x, and split the two fold trees across
engines so every engine sits under the ~200us SWDGE cast-DMA floor:
  ACT   : Square(kt) [128,8192], Ln, Exp, PSUM drains    (~140us)
  DVE   : prod=kt*qb8, num-fold 64->32, merged fold 16->8,
          reduce 8->1, mask-mult, relu*rk STT            (~160us)
  GPSIMD: SWDGE descgen, sq-folds 64->32->16, num 32->16 (~150us)
  PE    : [128,128] transposes of (t4,b,h) blocks        (~15us)
Mask is applied by multiplication (relu(num)*rk >= 0 so mask-mult gives
exact zeros), replacing v1's log-space -1e30 bias trick; this lets Ln/Exp
batch over 8 subtiles (bias operand is per-partition only).
Output staging: STT writes [128 keys, (t4,b,h)=128] blocks, PE transposes
them, ACT drains PSUM->SBUF, and 16 small HWDGE DMAs scatter to DRAM
(512B contiguous runs) incrementally instead of one end-of-kernel store.

Self-contained: only imports the platform libs from /opt/trn_rl_repo.
"""

import os
import sys

sys.path.insert(0, "/opt/trn_rl_repo")

import numpy as np

import concourse.bass as bass
import concourse.mybir as mybir
from concourse.tile import TileContext
from concourse.bass_utils import run_bass_kernel_spmd

# Keep the number of active DMA completion-sem lanes low: the kernel-tail
# Drain waits on every active proc's semaphore and walrus rejects
# instructions with too many sync waits.
import concourse.tile_sem_assignment as _tsa

_tsa.NUM_HWDGE_SEMS = 2
_tsa.NUM_SWDGE_GLOBAL_SEMS = 2

# The walrus build in this environment accepts at most ONE sync wait per
# instruction. Tile's scheduler can emit several (cross-engine RAW + WAR +
# DMA-lane waits). Splitting the extra waits into standalone EventSemaphore
# instructions on the same engine is semantically identical: the engine's
# sequencer executes them in order immediately before the instruction.
import orjson as _orjson


def _split_multi_waits(bir_bytes: bytes) -> bytes:
    m = _orjson.loads(bir_bytes)
    changed = False
    for fn in m.get("functions", []):
        for bb in fn.get("blocks", []):
            insts = bb.get("instructions")
            if not insts:
                continue
            out_list = []
            for inst in insts:
                si = inst.get("sync_info")
                waits = (si or {}).get("on_wait") or []
                if len(waits) > 1:
                    changed = True
                    for k, w in enumerate(waits[:-1]):
                        out_list.append(
                            {
                                "debug": inst.get("debug", 0),
                                "engine": inst["engine"],
                                "ins": [],
                                "name": f"{inst['name']}_wsplit{k}",
                                "opcode": "EventSemaphore",
                                "outs": [],
                                "sync_info": {"on_update": [], "on_wait": [w]},
                            }
                        )
                    si["on_wait"] = [waits[-1]]
                out_list.append(inst)
            bb["instructions"] = out_list
    return _orjson.dumps(m) if changed else bir_bytes


_orig_to_json_bytes = bass.Bass.to_json_bytes


def _patched_to_json_bytes(self, *a, **kw):
    return _split_multi_waits(_orig_to_json_bytes(self, *a, **kw))


bass.Bass.to_json_bytes = _patched_to_json_bytes

F32 = mybir.dt.float32
BF16 = mybir.dt.bfloat16
I32 = mybir.dt.int32
Alu = mybir.AluOpType
Act = mybir.ActivationFunctionType
AX = mybir.AxisListType

H = 16      # heads
DK = 64     # head dim
DM = 1024   # d_model
P = 128     # SBUF partitions
N_CORES = 8

PRECISION = "bf16"  # key-stream compute dtype


def build_nc(n_batch: int, lk: int) -> bass.Bass:
    """Build the per-core Bass program.

    Per-core DRAM I/O:
      key   [n_batch, lk, 1024] f32    (shard of the key tensor)
      qb    [n_batch, 128, 1024] bf16  (host-broadcast qtilde rows)
      maskb [n_batch, 128, lk/128, 16] bf16 (mask bcast over heads,
                                        key l split as l = t*128 + p)
      ident [128, 128] f32
      out   [n_batch, 16, lk] f32
    """
    assert n_batch == 2, "kernel assumes a batch pair per core"
    cdt = BF16
    ntiles = lk // P            # 128-key subtiles per batch
    TG = min(8, ntiles)         # subtiles per DMA group
    ngroups = ntiles // TG
    GW = TG * DM                # group width in elements (8192)
    TH = TG * H                 # (t, h) composite segments per group (128)

    nc = bass.Bass()
    key_in = nc.declare_dram_parameter("key", [n_batch, lk, DM], F32, isOutput=False)
    qb_in = nc.declare_dram_parameter("qb", [n_batch, P, DM], cdt, isOutput=False)
    mask_in = nc.declare_dram_parameter(
        "maskb", [n_batch, P, ntiles, H], cdt, isOutput=False
    )
    ident_in = nc.declare_dram_parameter("ident", [P, P], F32, isOutput=False)
    out = nc.declare_dram_parameter("out", [n_batch, H, lk], F32, isOutput=True)
    # DRAM views for per-64-partition-block stores: partition = (t4, h),
    # columns = keys l within the 4-subtile block, one view per batch.
    # out[b, h, (g4*4 + t4)*128 + l]  <-  view[b][t4, h, g4, l]
    out_vs = [
        out[b].rearrange("h (g t l) -> t h g l", t=4, l=P)
        for b in range(n_batch)
    ]

    def fold(eng, out_ap, in_ap, width, win):
        """One fold level: view `in_ap` (width elems) as [P, S, win] and add
        its halves into `out_ap` [P, S, win//2]. Pairs are win//2 columns
        apart (<= 32B for win <= 32), which keeps both DVE read streams in
        one SBUF line — ~2x faster than the naive (d, d+32) pairing.
        Summation order is irrelevant: any pairing tree gives the same sums.
        """
        i3 = in_ap.rearrange("p (s d) -> p s d", d=win)
        eng.tensor_tensor(
            out_ap.rearrange("p (s d) -> p s d", d=win // 2),
            i3[:, :, 0 : win // 2],
            i3[:, :, win // 2 : win],
            Alu.add,
        )

    with TileContext(nc) as tc:
        with (
            tc.tile_pool(name="const", bufs=1) as cpool,
            tc.tile_pool(name="kbig", bufs=3) as kpool,
            tc.tile_pool(name="sqp", bufs=2) as qpool,
            tc.tile_pool(name="work", bufs=1) as wpool,
            tc.tile_pool(name="sf1p", bufs=2) as fpool,
            tc.tile_pool(name="small", bufs=2) as spool,
            tc.tile_pool(name="ppp", bufs=3) as ppool,
            tc.tile_pool(name="ocp", bufs=2) as opool,
            tc.tile_pool(name="psum", bufs=4, space="PSUM") as pspool,
        ):
            def emit_loads(g):
                kts = []
                for b in range(n_batch):
                    kt = kpool.tile([P, GW + 128], cdt, name="kt", tag=f"kt{b}")
                    src = key_in[b].rearrange("(t p) c -> p t c", p=P)[
                        :, g * TG : (g + 1) * TG, :
                    ]
                    dst = kt[:, 0:GW].rearrange("p (t c) -> p t c", c=DM)
                    nc.gpsimd.dma_start(out=dst, in_=src)  # casts f32->bf16
                    kts.append(kt)
                return kts

            # group-0 key loads first: the DMA stream is the span floor
            kts = emit_loads(0)

            # --- constants ---
            ident = cpool.tile([P, P], F32, name="ident_r")
            nc.gpsimd.dma_start(out=ident[:], in_=ident_in[:])

            qb4s, maskbs = [], []
            for b in range(n_batch):
                qb_r = cpool.tile([P, DM], cdt, name=f"qbr{b}")
                nc.gpsimd.dma_start(out=qb_r[:], in_=qb_in[b])
                # 4x-tiled qtilde rows; both 4-subtile halves of a group
                # multiply against the same tile
                qb4 = cpool.tile([P, GW // 2], cdt, name=f"qb4_{b}")
                for t in range(TG // 2):
                    nc.scalar.copy(qb4[:, t * DM : (t + 1) * DM], qb_r[:])
                qb4s.append(qb4)
                mb = cpool.tile([P, ntiles * H], cdt, name=f"maskb{b}")
                nc.gpsimd.dma_start(
                    out=mb.rearrange("p (t h) -> p t h", h=H), in_=mask_in[b]
                )
                maskbs.append(mb)

            for g in range(ngroups):
                next_kts = emit_loads(g + 1) if g + 1 < ngroups else None
                pps = [
                    ppool.tile([P, P], F32, name="pp4", tag=f"pp4_{j}")
                    for j in range(2)
                ]
                for b in range(n_batch):
                    kt = kts[b]
                    # squares in two halves (ACT) for pipelining into GPSIMD
                    sqs = []
                    for u in range(2):
                        sqh = qpool.tile([P, GW // 2 + 128], cdt, name="sq", tag="sq")
                        nc.scalar.activation(
                            sqh[:, 0 : GW // 2],
                            kt[:, u * GW // 2 : (u + 1) * GW // 2],
                            Act.Square,
                        )
                        sqs.append(sqh)
                    # q*k products (DVE), whole group, two half-muls against
                    # the shared qb4 tile
                    prod = wpool.tile([P, GW + 128], cdt, name="prod", tag="prod")
                    for u in range(2):
                        nc.vector.tensor_tensor(
                            prod[:, u * GW // 2 : (u + 1) * GW // 2],
                            kt[:, u * GW // 2 : (u + 1) * GW // 2],
                            qb4s[b][:],
                            Alu.mult,
                        )

                    # num fold L1 (DVE); sq fold L1 (GPSIMD, per half)
                    nh1 = wpool.tile([P, GW // 2], cdt, name="nh1", tag="nh1")
                    fold(nc.vector, nh1[:], prod[:, 0:GW], GW, 32)
                    sf1 = fpool.tile([P, GW // 2 + 128], cdt, name="sf1", tag="sf1")
                    for u in range(2):
                        fold(
                            nc.gpsimd,
                            sf1[:, u * GW // 4 : (u + 1) * GW // 4],
                            sqs[u][:, 0 : GW // 2],
                            GW // 2,
                            32,
                        )

                    # fold L2 (DVE) into one shared tile:
                    # [0:GW//4] = num survivors, [GW//4:GW//2] = sq survivors
                    sh2 = wpool.tile([P, GW // 2 + 128], cdt, name="sh2", tag="sh2")
                    fold(nc.vector, sh2[:, 0 : GW // 4], nh1[:], GW // 2, 16)
                    fold(nc.vector, sh2[:, GW // 4 : GW // 2],
                         sf1[:, 0 : GW // 2], GW // 2, 16)

                    # merged fold L3 (DVE), then reduce 8 -> 1 (DVE)
                    f3 = wpool.tile([P, GW // 4], cdt, name="f3", tag="f3")
                    fold(nc.vector, f3[:], sh2[:, 0 : GW // 2], GW // 2, 8)
                    f4 = spool.tile([P, GW // 8], cdt, name="f4", tag="f4")
                    fold(nc.vector, f4[:], f3[:], GW // 4, 8)
                    f5 = spool.tile([P, GW // 16], cdt, name="f5", tag="f5")
                    fold(nc.vector, f5[:], f4[:], GW // 8, 4)
                    ns2 = spool.tile([P, 2 * TH], F32, name="ns2", tag="ns2")
                    fold(nc.vector, ns2[:], f5[:], GW // 16, 2)

                    # rk = exp(-0.5*ln(s2)) = 1/kn and relu(num) (ACT);
                    # mask-mult and final products (GPSIMD)
                    lns = spool.tile([P, TH], F32, name="lns", tag="lns")
                    nc.scalar.activation(lns[:], ns2[:, TH : 2 * TH], Act.Ln)
                    rk = spool.tile([P, TH], F32, name="rk", tag="rk")
                    nc.scalar.activation(rk[:], lns[:], Act.Exp, scale=-0.5)
                    numr = spool.tile([P, TH], F32, name="numr", tag="numr")
                    nc.scalar.activation(numr[:], ns2[:, 0:TH], Act.Relu)
                    rkm = spool.tile([P, TH], F32, name="rkm", tag="rkm")
                    nc.gpsimd.tensor_tensor(
                        rkm[:], rk[:], maskbs[b][:, g * TH : (g + 1) * TH], Alu.mult
                    )

                    # pp[j][:, b*64 + (t4, h)] = relu(num) * rkm  (contiguous)
                    for j in range(2):
                        sl = slice(j * TH // 2, (j + 1) * TH // 2)
                        nc.gpsimd.tensor_tensor(
                            pps[j][:, b * TH // 2 : (b + 1) * TH // 2],
                            numr[:, sl],
                            rkm[:, sl],
                            Alu.mult,
                        )
                # transpose to (b,t4,h) partitions and store incrementally
                for j in range(2):
                    g4 = 2 * g + j
                    tp = pspool.tile([P, P], F32, name="tp", tag="tp")
                    nc.tensor.transpose(tp[:], pps[j][:], ident[:])
                    oc = opool.tile([P, P], F32, name="oc", tag="oc")
                    nc.scalar.copy(oc[:], tp[:])
                    for b in range(n_batch):
                        nc.sync.dma_start(
                            out=out_vs[b][:, :, g4, :],
                            in_=oc[b * TH // 2 : (b + 1) * TH // 2, :],
                        )
                kts = next_kts
    return nc


_NC_CACHE: dict = {}


def _get_nc(n_batch, lk, precision=PRECISION):
    key = (n_batch, lk, precision)
    if key not in _NC_CACHE:
        _NC_CACHE[key] = build_nc(n_batch, lk)
    return _NC_CACHE[key]


def _host_prep(query, key, mask):
    """qtilde (qn and 1/Lk folded) broadcast rows + head-broadcast mask."""
    B, lk, dm = key.shape
    assert dm == DM
    cdt_np = mybir.dt.np(BF16)

    q = query.reshape(B, H, DK).astype(np.float64)
    qn = np.sqrt((q * q).sum(-1))  # [B, H]
    qt = q / (qn[:, :, None] * float(lk))  # qtilde [B, H, DK]
    qb = np.ascontiguousarray(
        np.broadcast_to(qt.reshape(B, 1, DM), (B, P, DM))
    ).astype(cdt_np)

    ntiles = lk // P
    # maskb[b, p, t, h] = mask[b, t*128 + p]
    mb = mask.reshape(B, ntiles, P).transpose(0, 2, 1)[:, :, :, None]
    maskb = np.ascontiguousarray(
        np.broadcast_to(mb, (B, P, ntiles, H))
    ).astype(cdt_np)
    return qb, maskb


def prep_inputs(query, key, mask, n_cores=N_CORES):
    """Shard + host-side input prep (per-core in_maps for CoreSim/native)."""
    B = key.shape[0]
    nb = B // n_cores
    qb, maskb = _host_prep(query, key, mask)
    ident = np.eye(P, dtype=np.float32)
    in_maps = []
    for c in range(n_cores):
        sl = slice(c * nb, (c + 1) * nb)
        in_maps.append(
            {
                "key": np.ascontiguousarray(key[sl]),
                "qb": qb[sl],
                "maskb": maskb[sl],
                "ident": ident,
            }
        )
    return in_maps


class _Runner:
    """Cached PJRT executable for one built Bass program.

    Mirrors bass2jax.run_bass_via_pjrt but jits ONCE, and feeds the
    global (unsharded) arrays directly: shard_map splits axis 0 across
    the 8 cores, which is exactly the per-core batch shard.
    """

    def __init__(self, nc, n_cores):
        import jax
        from jax.sharding import Mesh, PartitionSpec
        from jax.experimental.shard_map import shard_map
        from concourse import bass2jax as b2j

        b2j.install_neuronx_cc_hook()
        self.jax = jax
        self.n_cores = n_cores
        part_name = (
            nc.partition_id_tensor.name if nc.partition_id_tensor else None
        )
        in_names, out_names, out_avals, zero_outs = [], [], [], []
        for alloc in nc.m.functions[0].allocations:
            if not isinstance(alloc, mybir.MemoryLocationSet):
                continue
            name = alloc.memorylocations[0].name
            if alloc.kind == "ExternalInput":
                if name != part_name:
                    in_names.append(name)
            elif alloc.kind == "ExternalOutput":
                out_names.append(name)
                shape = tuple(alloc.tensor_shape)
                dtype = mybir.dt.np(alloc.dtype)
                out_avals.append(jax.core.ShapedArray(shape, dtype))
                zero_outs.append(np.zeros(shape, dtype))
        self.in_names, self.out_names = in_names, out_names
        self.out_avals, self.zero_outs = out_avals, zero_outs
        n_params, n_outs = len(in_names), len(out_names)

        bind_in_names = in_names + out_names
        if part_name is not None:
            bind_in_names = bind_in_names + [part_name]

        def _body(*args):
            operands = list(args)
            if part_name is not None:
                operands.append(b2j.partition_id_tensor())
            outs = b2j._bass_exec_p.bind(
                *operands,
                out_avals=tuple(out_avals),
                in_names=tuple(bind_in_names),
                out_names=tuple(out_names),
                lowering_input_output_aliases=(),
                sim_require_finite=True,
                sim_require_nnan=True,
                nc=nc,
            )
            return tuple(outs)

        devices = jax.devices()[:n_cores]
        self.mesh = Mesh(np.asarray(devices), ("core",))
        in_specs = (PartitionSpec("core"),) * (n_params + n_outs)
        out_specs = (PartitionSpec("core"),) * n_outs
        self.fn = jax.jit(
            shard_map(
                _body,
                mesh=self.mesh,
                in_specs=in_specs,
                out_specs=out_specs,
                check_rep=False,
            ),
            donate_argnums=tuple(range(n_params, n_params + n_outs)),
            keep_unused=True,
        )

    def global_args(self, global_ins: dict):
        args = [global_ins[name] for name in self.in_names]
        args += [
            np.zeros((self.n_cores * z.shape[0], *z.shape[1:]), z.dtype)
            for z in self.zero_outs
        ]
        return args

    def __call__(self, global_ins: dict):
        out_arrs = self.fn(*self.global_args(global_ins))
        return {
            name: np.asarray(out_arrs[i]) for i, name in enumerate(self.out_names)
        }


_RUNNER_CACHE: dict = {}


def _get_runner(n_batch, lk, precision=PRECISION):
    key = (n_batch, lk, precision)
    if key not in _RUNNER_CACHE:
        nc = _get_nc(n_batch, lk, precision)
        if not nc.is_finalized():
            nc.finalize()
        _RUNNER_CACHE[key] = _Runner(nc, N_CORES)
    return _RUNNER_CACHE[key]


def global_inputs(query, key, mask):
    """Host prep producing the UNSHARDED arrays fed to shard_map (axis 0
    splits evenly across the 8 cores == batch sharding). Zero-copy for key."""
    qb, maskb = _host_prep(query, key, mask)
    ident = np.tile(np.eye(P, dtype=np.float32), (N_CORES, 1)).reshape(
        N_CORES * P, P
    )
    return {"key": np.ascontiguousarray(key), "qb": qb, "maskb": maskb,
            "ident": ident}


def kernel(query, key, mask, trace=False):
    B, lk, _ = key.shape
    nb = B // N_CORES
    runner = _get_runner(nb, lk)
    gins = global_inputs(query, key, mask)
    out = runner(gins)["out"]  # [B, H, lk] concat over cores on axis 0
    full = out.reshape(B, H, lk)
    return full


if __name__ == "__main__":
    # smoke test at reduced size
    rng = np.random.default_rng(0)
    B, lk = 16, 1024
    query = rng.standard_normal((B, 1, DM), dtype=np.float32)
    key = rng.standard_normal((B, lk, DM), dtype=np.float32)
    mask = rng.integers(0, 2, (B, lk)).astype(np.int32)
    out = kernel(query, key, mask)
    print("out", out.shape, out.dtype, float(np.abs(out).max()))


# revision 3
# speedup vs baseline: 266.5886x; 1.1636x over previous
"""Trainium2 Bass kernel for masked cosine-similarity attention scores.

Problem: nn_MultiHeadedAttention_2 (sparse_attention, memory-bound)
  query [16, 1, 1024] f32, key [16, 8192, 1024] f32, mask [16, 8192] int32
  out   [16, 16, 8192] f32 = relu(cos_sim_per_head(q, k) masked) / Lk

Math (per batch b, head h, key position l):
  num[h,l] = sum_d qtilde[h,d] * k[l, h*64+d]     (qtilde = q/(qn*Lk), host)
  s2[h,l]  = sum_d k[l, h*64+d]^2
  p        = max(num, 0) * exp(-0.5*ln(s2)) * mask[l]
  The reference's EPS=1e-8 guard on qn*kn is unreachable for randn inputs.

Sharding: data-parallel over batch B=16 -> 2 batches per core x 8 cores.

Design (vs the first kernel, which issued ~1800 tiny instructions and ran
430us/core, DVE-bound): batch every op over a whole 8-subtile DMA group (1024 keys)
so instruction count drops ~6x, and split the two fold trees across
engines. GPSIMD is kept nearly idle (descgen + two tiny muls): its
DSP-style SBUF traffic during big folds was measured to stall concurrent
DVE ops by 1.5-2.7x, costing more than it contributed.
  ACT   : Square(kt), Ln, Exp, Relu, PSUM drains         (~165us)
  DVE   : prod, BOTH fold trees, tail folds              (~245us)
  GPSIMD: SWDGE descgen, mask-mult, final products       (~35us)
  PE    : [128,128] transposes of (b,t4,h) blocks        (~15us)
Mask is applied by multiplication (relu(num)*rk >= 0 so mask-mult gives
exact zeros), replacing v1's log-space -1e30 bias trick; this lets Ln/Exp
batch over 8 subtiles (bias operand is per-partition only).
Output staging: STT writes [128 keys, (t4,b,h)=128] blocks, PE transposes
them, ACT drains PSUM->SBUF, and 16 small HWDGE DMAs scatter to DRAM
(512B contiguous runs) incrementally instead of one end-of-kernel store.

Self-contained: only imports the platform libs from /opt/trn_rl_repo.
"""

import os
import sys

sys.path.insert(0, "/opt/trn_rl_repo")

import numpy as np

import concourse.bass as bass
import concourse.mybir as mybir
from concourse.tile import TileContext
from concourse.bass_utils import run_bass_kernel_spmd

# Keep the number of active DMA completion-sem lanes low: the kernel-tail
# Drain waits on every active proc's semaphore and walrus rejects
# instructions with too many sync waits.
import concourse.tile_sem_assignment as _tsa

_tsa.NUM_HWDGE_SEMS = 2
_tsa.NUM_SWDGE_GLOBAL_SEMS = 2

# The walrus build in this environment accepts at most ONE sync wait per
# instruction. Tile's scheduler can emit several (cross-engine RAW + WAR +
# DMA-lane waits). Splitting the extra waits into standalone EventSemaphore
# instructions on the same engine is semantically identical: the engine's
# sequencer executes them in order immediately before the instruction.
import orjson as _orjson


def _split_multi_waits(bir_bytes: bytes) -> bytes:
    m = _orjson.loads(bir_bytes)
    changed = False
    for fn in m.get("functions", []):
        for bb in fn.get("blocks", []):
            insts = bb.get("instructions")
            if not insts:
                continue
            out_list = []
            for inst in insts:
                si = inst.get("sync_info")
                waits = (si or {}).get("on_wait") or []
                if len(waits) > 1:
                    changed = True
                    for k, w in enumerate(waits[:-1]):
                        out_list.append(
                            {
                                "debug": inst.get("debug", 0),
                                "engine": inst["engine"],
                                "ins": [],
                                "name": f"{inst['name']}_wsplit{k}",
                                "opcode": "EventSemaphore",
                                "outs": [],
                                "sync_info": {"on_update": [], "on_wait": [w]},
                            }
                        )
                    si["on_wait"] = [waits[-1]]
                out_list.append(inst)
            bb["instructions"] = out_list
    return _orjson.dumps(m) if changed else bir_bytes


_orig_to_json_bytes = bass.Bass.to_json_bytes


def _patched_to_json_bytes(self, *a, **kw):
    return _split_multi_waits(_orig_to_json_bytes(self, *a, **kw))


bass.Bass.to_json_bytes = _patched_to_json_bytes

F32 = mybir.dt.float32
BF16 = mybir.dt.bfloat16
I32 = mybir.dt.int32
Alu = mybir.AluOpType
Act = mybir.ActivationFunctionType
AX = mybir.AxisListType

H = 16      # heads
DK = 64     # head dim
DM = 1024   # d_model
P = 128     # SBUF partitions
N_CORES = 8

PRECISION = "bf16"  # key-stream compute dtype


def build_nc(n_batch: int, lk: int) -> bass.Bass:
    """Build the per-core Bass program.

    Per-core DRAM I/O:
      key   [n_batch, lk, 1024] f32    (shard of the key tensor)
      qb    [n_batch, 128, 1024] bf16  (host-broadcast qtilde rows)
      maskb [n_batch, 128, lk/128, 16] bf16 (mask bcast over heads,
                                        key l split as l = t*128 + p)
      ident [128, 128] f32
      out   [n_batch, 16, lk] f32
    """
    assert n_batch == 2, "kernel assumes a batch pair per core"
    cdt = BF16
    ntiles = lk // P            # 128-key subtiles per batch
    TG = min(8, ntiles)         # subtiles per DMA group
    ngroups = ntiles // TG
    GW = TG * DM                # group width in elements (8192)
    TH = TG * H                 # (t, h) composite segments per group (128)

    nc = bass.Bass()
    key_in = nc.declare_dram_parameter("key", [n_batch, lk, DM], F32, isOutput=False)
    qb_in = nc.declare_dram_parameter("qb", [n_batch, P, DM], cdt, isOutput=False)
    mask_in = nc.declare_dram_parameter(
        "maskb", [n_batch, P, ntiles, H], cdt, isOutput=False
    )
    ident_in = nc.declare_dram_parameter("ident", [P, P], F32, isOutput=False)
    out = nc.declare_dram_parameter("out", [n_batch, H, lk], F32, isOutput=True)
    # DRAM views for per-64-partition-block stores: partition = (t4, h),
    # columns = keys l within the 4-subtile block, one view per batch.
    # out[b, h, (g4*4 + t4)*128 + l]  <-  view[b][t4, h, g4, l]
    out_vs = [
        out[b].rearrange("h (g t l) -> t h g l", t=4, l=P)
        for b in range(n_batch)
    ]

    def fold(eng, out_ap, in_ap, width, win):
        """One fold level: view `in_ap` (width elems) as [P, S, win] and add
        its halves into `out_ap` [P, S, win//2]. Pairs are win//2 columns
        apart (<= 32B for win <= 32), which keeps both DVE read streams in
        one SBUF line — ~2x faster than the naive (d, d+32) pairing.
        Summation order is irrelevant: any pairing tree gives the same sums.
        """
        i3 = in_ap.rearrange("p (s d) -> p s d", d=win)
        eng.tensor_tensor(
            out_ap.rearrange("p (s d) -> p s d", d=win // 2),
            i3[:, :, 0 : win // 2],
            i3[:, :, win // 2 : win],
            Alu.add,
        )

    with TileContext(nc) as tc:
        with (
            tc.tile_pool(name="const", bufs=1) as cpool,
            tc.tile_pool(name="kbig", bufs=3) as kpool,
            tc.tile_pool(name="sqp", bufs=2) as qpool,
            tc.tile_pool(name="work", bufs=1) as wpool,
            tc.tile_pool(name="sf1p", bufs=2) as fpool,
            tc.tile_pool(name="small", bufs=2) as spool,
            tc.tile_pool(name="ppp", bufs=3) as ppool,
            tc.tile_pool(name="ocp", bufs=2) as opool,
            tc.tile_pool(name="psum", bufs=4, space="PSUM") as pspool,
        ):
            def emit_loads(g):
                kts = []
                for b in range(n_batch):
                    kt = kpool.tile([P, GW + 128], cdt, name="kt", tag=f"kt{b}")
                    src = key_in[b].rearrange("(t p) c -> p t c", p=P)[
                        :, g * TG : (g + 1) * TG, :
                    ]
                    dst = kt[:, 0:GW].rearrange("p (t c) -> p t c", c=DM)
                    nc.gpsimd.dma_start(out=dst, in_=src)  # casts f32->bf16
                    kts.append(kt)
                return kts

            # group-0 key loads first: the DMA stream is the span floor
            kts = emit_loads(0)

            # --- constants ---
            ident = cpool.tile([P, P], F32, name="ident_r")
            nc.gpsimd.dma_start(out=ident[:], in_=ident_in[:])

            qb4s, maskbs = [], []
            for b in range(n_batch):
                qb_r = cpool.tile([P, DM], cdt, name=f"qbr{b}")
                nc.gpsimd.dma_start(out=qb_r[:], in_=qb_in[b])
                # 4x-tiled qtilde rows; both 4-subtile halves of a group
                # multiply against the same tile
                qb4 = cpool.tile([P, GW // 2], cdt, name=f"qb4_{b}")
                for t in range(TG // 2):
                    nc.scalar.copy(qb4[:, t * DM : (t + 1) * DM], qb_r[:])
                qb4s.append(qb4)
                mb = cpool.tile([P, ntiles * H], cdt, name=f"maskb{b}")
                nc.gpsimd.dma_start(
                    out=mb.rearrange("p (t h) -> p t h", h=H), in_=mask_in[b]
                )
                maskbs.append(mb)

            for g in range(ngroups):
                next_kts = emit_loads(g + 1) if g + 1 < ngroups else None
                pps = [
                    ppool.tile([P, P], F32, name="pp4", tag=f"pp4_{j}")
                    for j in range(2)
                ]
                for b in range(n_batch):
                    kt = kts[b]
                    # squares in two halves (ACT) for pipelining into GPSIMD
                    sqs = []
                    for u in range(2):
                        sqh = qpool.tile([P, GW // 2 + 128], cdt, name="sq", tag="sq")
                        nc.scalar.activation(
                            sqh[:, 0 : GW // 2],
                            kt[:, u * GW // 2 : (u + 1) * GW // 2],
                            Act.Square,
                        )
                        sqs.append(sqh)
                    # q*k products (DVE), whole group, two half-muls against
                    # the shared qb4 tile
                    prod = wpool.tile([P, GW + 128], cdt, name="prod", tag="prod")
                    for u in range(2):
                        nc.vector.tensor_tensor(
                            prod[:, u * GW // 2 : (u + 1) * GW // 2],
                            kt[:, u * GW // 2 : (u + 1) * GW // 2],
                            qb4s[b][:],
                            Alu.mult,
                        )

                    # num fold L1 (DVE); sq fold L1 (GPSIMD, per half)
                    nh1 = wpool.tile([P, GW // 2], cdt, name="nh1", tag="nh1")
                    fold(nc.vector, nh1[:], prod[:, 0:GW], GW, 32)
                    sf1 = fpool.tile([P, GW // 2 + 128], cdt, name="sf1", tag="sf1")
                    for u in range(2):
                        fold(
                            nc.vector,
                            sf1[:, u * GW // 4 : (u + 1) * GW // 4],
                            sqs[u][:, 0 : GW // 2],
                            GW // 2,
                            32,
                        )

                    # fold L2 (DVE) into one shared tile:
                    # [0:GW//4] = num survivors, [GW//4:GW//2] = sq survivors
                    sh2 = wpool.tile([P, GW // 2 + 128], cdt, name="sh2", tag="sh2")
                    fold(nc.vector, sh2[:, 0 : GW // 4], nh1[:], GW // 2, 16)
                    fold(nc.vector, sh2[:, GW // 4 : GW // 2],
                         sf1[:, 0 : GW // 2], GW // 2, 16)

                    # merged fold L3 (DVE), then reduce 8 -> 1 (DVE)
                    f3 = wpool.tile([P, GW // 4], cdt, name="f3", tag="f3")
                    fold(nc.vector, f3[:], sh2[:, 0 : GW // 2], GW // 2, 8)
                    f4 = spool.tile([P, GW // 8], cdt, name="f4", tag="f4")
                    fold(nc.vector, f4[:], f3[:], GW // 4, 8)
                    f5 = spool.tile([P, GW // 16], cdt, name="f5", tag="f5")
                    fold(nc.vector, f5[:], f4[:], GW // 8, 4)
                    ns2 = spool.tile([P, 2 * TH], F32, name="ns2", tag="ns2")
                    fold(nc.vector, ns2[:], f5[:], GW // 16, 2)

                    # rk = exp(-0.5*ln(s2)) = 1/kn and relu(num) (ACT);
                    # mask-mult and final products (GPSIMD)
                    lns = spool.tile([P, TH], F32, name="lns", tag="lns")
                    nc.scalar.activation(lns[:], ns2[:, TH : 2 * TH], Act.Ln)
                    rk = spool.tile([P, TH], F32, name="rk", tag="rk")
                    nc.scalar.activation(rk[:], lns[:], Act.Exp, scale=-0.5)
                    numr = spool.tile([P, TH], F32, name="numr", tag="numr")
                    nc.scalar.activation(numr[:], ns2[:, 0:TH], Act.Relu)
                    rkm = spool.tile([P, TH], F32, name="rkm", tag="rkm")
                    nc.gpsimd.tensor_tensor(
                        rkm[:], rk[:], maskbs[b][:, g * TH : (g + 1) * TH], Alu.mult
                    )

                    # pp[j][:, b*64 + (t4, h)] = relu(num) * rkm  (contiguous)
                    for j in range(2):
                        sl = slice(j * TH // 2, (j + 1) * TH // 2)
                        nc.gpsimd.tensor_tensor(
                            pps[j][:, b * TH // 2 : (b + 1) * TH // 2],
                            numr[:, sl],
                            rkm[:, sl],
                            Alu.mult,
                        )
                # transpose to (b,t4,h) partitions and store incrementally
                for j in range(2):
                    g4 = 2 * g + j
                    tp = pspool.tile([P, P], F32, name="tp", tag="tp")
                    nc.tensor.transpose(tp[:], pps[j][:], ident[:])
                    oc = opool.tile([P, P], F32, name="oc", tag="oc")
                    nc.scalar.copy(oc[:], tp[:])
                    for b in range(n_batch):
                        nc.sync.dma_start(
                            out=out_vs[b][:, :, g4, :],
                            in_=oc[b * TH // 2 : (b + 1) * TH // 2, :],
                        )
                kts = next_kts
    return nc


_NC_CACHE: dict = {}


def _get_nc(n_batch, lk, precision=PRECISION):
    key = (n_batch, lk, precision)
    if key not in _NC_CACHE:
        _NC_CACHE[key] = build_nc(n_batch, lk)
    return _NC_CACHE[key]


def _host_prep(query, key, mask):
    """qtilde (qn and 1/Lk folded) broadcast rows + head-broadcast mask."""
    B, lk, dm = key.shape
    assert dm == DM
    cdt_np = mybir.dt.np(BF16)

    q = query.reshape(B, H, DK).astype(np.float64)
    qn = np.sqrt((q * q).sum(-1))  # [B, H]
    qt = q / (qn[:, :, None] * float(lk))  # qtilde [B, H, DK]
    qb = np.ascontiguousarray(
        np.broadcast_to(qt.reshape(B, 1, DM), (B, P, DM))
    ).astype(cdt_np)

    ntiles = lk // P
    # maskb[b, p, t, h] = mask[b, t*128 + p]
    mb = mask.reshape(B, ntiles, P).transpose(0, 2, 1)[:, :, :, None]
    maskb = np.ascontiguousarray(
        np.broadcast_to(mb, (B, P, ntiles, H))
    ).astype(cdt_np)
    return qb, maskb


def prep_inputs(query, key, mask, n_cores=N_CORES):
    """Shard + host-side input prep (per-core in_maps for CoreSim/native)."""
    B = key.shape[0]
    nb = B // n_cores
    qb, maskb = _host_prep(query, key, mask)
    ident = np.eye(P, dtype=np.float32)
    in_maps = []
    for c in range(n_cores):
        sl = slice(c * nb, (c + 1) * nb)
        in_maps.append(
            {
                "key": np.ascontiguousarray(key[sl]),
                "qb": qb[sl],
                "maskb": maskb[sl],
                "ident": ident,
            }
        )
    return in_maps


class _Runner:
    """Cached PJRT executable for one built Bass program.

    Mirrors bass2jax.run_bass_via_pjrt but jits ONCE, and feeds the
    global (unsharded) arrays directly: shard_map splits axis 0 across
    the 8 cores, which is exactly the per-core batch shard.
    """

    def __init__(self, nc, n_cores):
        import jax
        from jax.sharding import Mesh, PartitionSpec
        from jax.experimental.shard_map import shard_map
        from concourse import bass2jax as b2j

        b2j.install_neuronx_cc_hook()
        self.jax = jax
        self.n_cores = n_cores
        part_name = (
            nc.partition_id_tensor.name if nc.partition_id_tensor else None
        )
        in_names, out_names, out_avals, zero_outs = [], [], [], []
        for alloc in nc.m.functions[0].allocations:
            if not isinstance(alloc, mybir.MemoryLocationSet):
                continue
            name = alloc.memorylocations[0].name
            if alloc.kind == "ExternalInput":
                if name != part_name:
                    in_names.append(name)
            elif alloc.kind == "ExternalOutput":
                out_names.append(name)
                shape = tuple(alloc.tensor_shape)
                dtype = mybir.dt.np(alloc.dtype)
                out_avals.append(jax.core.ShapedArray(shape, dtype))
                zero_outs.append(np.zeros(shape, dtype))
        self.in_names, self.out_names = in_names, out_names
        self.out_avals, self.zero_outs = out_avals, zero_outs
        n_params, n_outs = len(in_names), len(out_names)

        bind_in_names = in_names + out_names
        if part_name is not None:
            bind_in_names = bind_in_names + [part_name]

        def _body(*args):
            operands = list(args)
            if part_name is not None:
                operands.append(b2j.partition_id_tensor())
            outs = b2j._bass_exec_p.bind(
                *operands,
                out_avals=tuple(out_avals),
                in_names=tuple(bind_in_names),
                out_names=tuple(out_names),
                lowering_input_output_aliases=(),
                sim_require_finite=True,
                sim_require_nnan=True,
                nc=nc,
            )
            return tuple(outs)

        devices = jax.devices()[:n_cores]
        self.mesh = Mesh(np.asarray(devices), ("core",))
        in_specs = (PartitionSpec("core"),) * (n_params + n_outs)
        out_specs = (PartitionSpec("core"),) * n_outs
        self.fn = jax.jit(
            shard_map(
                _body,
                mesh=self.mesh,
                in_specs=in_specs,
                out_specs=out_specs,
                check_rep=False,
            ),
            donate_argnums=tuple(range(n_params, n_params + n_outs)),
            keep_unused=True,
        )

    def global_args(self, global_ins: dict):
        args = [global_ins[name] for name in self.in_names]
        args += [
            np.zeros((self.n_cores * z.shape[0], *z.shape[1:]), z.dtype)
            for z in self.zero_outs
        ]
        return args

    def __call__(self, global_ins: dict):
        out_arrs = self.fn(*self.global_args(global_ins))
        return {
            name: np.asarray(out_arrs[i]) for i, name in enumerate(self.out_names)
        }


_RUNNER_CACHE: dict = {}


def _get_runner(n_batch, lk, precision=PRECISION):
    key = (n_batch, lk, precision)
    if key not in _RUNNER_CACHE:
        nc = _get_nc(n_batch, lk, precision)
        if not nc.is_finalized():
            nc.finalize()
        _RUNNER_CACHE[key] = _Runner(nc, N_CORES)
    return _RUNNER_CACHE[key]


def global_inputs(query, key, mask):
    """Host prep producing the UNSHARDED arrays fed to shard_map (axis 0
    splits evenly across the 8 cores == batch sharding). Zero-copy for key."""
    qb, maskb = _host_prep(query, key, mask)
    ident = np.tile(np.eye(P, dtype=np.float32), (N_CORES, 1)).reshape(
        N_CORES * P, P
    )
    return {"key": np.ascontiguousarray(key), "qb": qb, "maskb": maskb,
            "ident": ident}


def kernel(query, key, mask, trace=False):
    B, lk, _ = key.shape
    nb = B // N_CORES
    runner = _get_runner(nb, lk)
    gins = global_inputs(query, key, mask)
    out = runner(gins)["out"]  # [B, H, lk] concat over cores on axis 0
    full = out.reshape(B, H, lk)
    return full


if __name__ == "__main__":
    # smoke test at reduced size
    rng = np.random.default_rng(0)
    B, lk = 16, 1024
    query = rng.standard_normal((B, 1, DM), dtype=np.float32)
    key = rng.standard_normal((B, lk, DM), dtype=np.float32)
    mask = rng.integers(0, 2, (B, lk)).astype(np.int32)
    out = kernel(query, key, mask)
    print("out", out.shape, out.dtype, float(np.abs(out).max()))
